# revision 1
# baseline (speedup 1.0000x reference)
"""GCN block kernel for TRN2, 8-core SPMD.

Algorithm (per core, destination-sharded):
  out[dst] = relu( (sum_e norm_e * x[src_e]) @ W.T + bias ) + x[dst]
Reassociation: the edge aggregation runs on raw x rows (gathered via
dma_gather), accumulated per 128-row destination tile via one-hot matmuls
on the PE; a single 128x128 weight matmul per destination tile finishes
the job, with bias folded in as a rank-1 matmul.

Edge partitioning: by destination core/tile, sub-grouped by source window
(int16 gather index limit = 32768 rows). Uniform SPMD schedule: per-
(tile,window) groups padded to a shared capacity; trailing -1 indices are
skipped by the Q7 descriptor generator at ~zero cost.
"""
import sys
sys.path.insert(0, '/opt/trn_rl_repo')
import numpy as np
from contextlib import ExitStack

import concourse.bacc as bacc
import concourse.mybir as mybir
from concourse.library_config import mlp

F32 = mybir.dt.float32
I16 = mybir.dt.int16
ALU = mybir.AluOpType

D = 128


def roundup(x, m):
    return (x + m - 1) // m * m


class Cfg:
    def __init__(self, N, E, NC=8, WIN=32768, TB=8, NG=6, NP=32, NF=4, NEO=4):
        self.N, self.E, self.NC = N, E, NC
        assert N % NC == 0
        self.SHARD = N // NC
        self.TILES = (self.SHARD + 127) // 128  # dst tiles per core
        self.WIN = WIN
        self.NW = (N + WIN - 1) // WIN          # source windows
        self.TB = TB                            # dst tiles per batch
        self.NG = NG                            # gather ring depth
        self.NP = NP                            # P ring (chunks)
        self.NF = NF                            # final psum ring
        self.NEO = NEO                          # epilogue sbuf ring


def prep(cfg, x, weight, bias, edge_weight, edge_index):
    """Host preprocessing -> (schedule, per-core in_maps pieces, caps)."""
    N, NC, SHARD, TILES, WIN, NW = cfg.N, cfg.NC, cfg.SHARD, cfg.TILES, cfg.WIN, cfg.NW
    src = np.asarray(edge_index[0], dtype=np.int64)
    dst = np.asarray(edge_index[1], dtype=np.int64)
    ew = np.asarray(edge_weight, dtype=np.float64)
    deg = np.bincount(dst, weights=ew, minlength=N) + 1.0
    dinv = (1.0 / np.sqrt(deg)).astype(np.float32)
    norm = (dinv[src] * ew.astype(np.float32) * dinv[dst]).astype(np.float32)

    loop = np.arange(N, dtype=np.int64)
    a_src = np.concatenate([src, loop])
    a_dst = np.concatenate([dst, loop])
    a_nrm = np.concatenate([norm, (dinv * dinv).astype(np.float32)])

    core = a_dst // SHARD
    dstloc = a_dst - core * SHARD
    tile = dstloc // 128
    col = (dstloc % 128).astype(np.float32)
    win = a_src // WIN
    srcloc = (a_src - win * WIN).astype(np.int32)

    key = (core * TILES + tile) * NW + win
    order = np.argsort(key, kind='stable')
    k_sorted = key[order]
    counts = np.bincount(key, minlength=NC * TILES * NW).reshape(NC, TILES, NW)
    starts = np.zeros(NC * TILES * NW + 1, dtype=np.int64)
    np.cumsum(counts.reshape(-1), out=starts[1:])

    cap_w = [max(128, roundup(int(counts[:, :, w].max()), 128)) for w in range(NW)]

    s_srcloc = srcloc[order]
    s_nrm = a_nrm[order]
    s_col = col[order]

    # Call order: tile-major (each tile's NW windows consecutive) so PSUM
    # accumulation groups stay contiguous on the PE.
    n_batches = TILES
    sched = []   # per call dict
    for I in range(TILES):
        for w in range(NW):
            sched.append(dict(w=w, I=I, slot=I % 2,
                              first=(w == 0), last=(w == NW - 1)))

    # uniform valid count per call across cores: num_idxs_reg must equal the
    # actual non-negative count on every core (one SPMD program, one immediate)
    vq = []
    for q, call in enumerate(sched):
        mx = 1
        for c in range(NC):
            g = (c * TILES + call['I']) * NW + call['w']
            mx = max(mx, int(starts[g + 1] - starts[g]))
        vq.append(mx)

    # per-call capacity: vq rounded to a chunk — DVE/PE only touch real chunks
    for q, call in enumerate(sched):
        call['cap'] = roundup(vq[q], 128)
        call['CH'] = call['cap'] // 128
    icols = [c['cap'] // 16 for c in sched]
    chs = [c['CH'] for c in sched]
    icol_off = np.concatenate([[0], np.cumsum(icols)])
    ch_off = np.concatenate([[0], np.cumsum(chs)])
    ICOLS_TOT, CH_TOT = int(icol_off[-1]), int(ch_off[-1])

    idx_streams, nv_streams, dv_streams, counts_per_call = [], [], [], []
    for c in range(NC):
        idx_s = np.full((16, ICOLS_TOT), -1, dtype=np.int16)
        nd_s = np.zeros((128, 2 * CH_TOT), dtype=np.float32)
        ccnt = []
        for q, call in enumerate(sched):
            g = (c * TILES + call['I']) * NW + call['w']
            lo, hi = int(starts[g]), int(starts[g + 1])
            cnt = hi - lo
            cap = call['cap']
            assert cnt <= cap, (cnt, cap)
            if cnt == 0:
                # one harmless dummy so the gather isn't empty
                iv = np.zeros(1, dtype=np.int16)
                nv = np.zeros(1, dtype=np.float32)
                dvv = np.zeros(1, dtype=np.float32)
                cnt = 1
            else:
                iv = s_srcloc[lo:hi].astype(np.int16)
                nv = s_nrm[lo:hi]
                dvv = s_col[lo:hi]
            ccnt.append(cnt)
            ipad = np.full(cap, -1, dtype=np.int16)
            ipad[:vq[q]] = 0
            ipad[:cnt] = iv
            idx_s[:, icol_off[q]:icol_off[q + 1]] = ipad.reshape(-1, 16).T
            npad = np.zeros(cap, dtype=np.float32); npad[:cnt] = nv
            dpad = np.zeros(cap, dtype=np.float32); dpad[:cnt] = dvv
            ch = cap // 128
            nd_s[:, 2 * ch_off[q]:2 * ch_off[q] + ch] = npad.reshape(-1, 128).T
            nd_s[:, 2 * ch_off[q] + ch:2 * ch_off[q + 1]] = dpad.reshape(-1, 128).T
        idx_streams.append(np.tile(idx_s, (8, 1)))
        nv_streams.append(nd_s)
        counts_per_call.append(ccnt)

    meta = dict(sched=sched, cap_w=cap_w, icol_off=icol_off, ch_off=ch_off, vq=vq,
                ICOLS_TOT=ICOLS_TOT, CH_TOT=CH_TOT, n_batches=n_batches,
                counts_per_call=counts_per_call)

    wt = np.ascontiguousarray(np.asarray(weight, dtype=np.float32).T)  # wt[k,o]=W[o,k]
    bias_row = np.asarray(bias, dtype=np.float32).reshape(1, D)
    ones_row = np.ones((1, D), dtype=np.float32)
    iota = np.tile(np.arange(D, dtype=np.float32), (128, 1))

    xf = np.asarray(x, dtype=np.float32)
    in_maps = []
    for c in range(NC):
        in_maps.append({
            "xfull": xf,
            "xshard": np.ascontiguousarray(xf[c * SHARD:(c + 1) * SHARD]),
            "idxs": idx_streams[c],
            "nds": nv_streams[c],
            "wt": wt, "bias_row": bias_row, "ones_row": ones_row, "iota": iota,
        })
    return meta, in_maps


def build(cfg, meta, sim_core=None, strip=None, reps=1):
    """Build the SPMD program. sim_core: if set, use that core's exact
    per-call counts as num_idxs_reg (CoreSim validation)."""
    N, SHARD, TILES, WIN, NW = cfg.N, cfg.SHARD, cfg.TILES, cfg.WIN, cfg.NW
    sched, icol_off, ch_off = meta['sched'], meta['icol_off'], meta['ch_off']
    NCALLS = len(sched)
    CHmax = max(c['CH'] for c in sched)
    ICOLmax = max(c['cap'] // 16 for c in sched)

    # Flatten reps x sched into one global schedule with global call index q,
    # global tile index t, and dram_q for DRAM stream offsets.
    gsched = []
    for r in range(reps):
        for q, call in enumerate(sched):
            gsched.append(dict(call, dram_q=q, t=r * TILES + call['I']))
    GT = reps * TILES   # total global tiles
    cum_chunks = np.concatenate([[0], np.cumsum([c['CH'] for c in gsched])])
    tile_last_chunk = {}
    for q, call in enumerate(gsched):
        if call['last']:
            tile_last_chunk[call['t']] = int(cum_chunks[q + 1]) - 1

    nc = bacc.Bacc("TRN2", num_swdge_queues=4)

    xfull = nc.dram_tensor("xfull", [N, D], F32, kind="ExternalInput")
    xshard = nc.dram_tensor("xshard", [SHARD, D], F32, kind="ExternalInput")
    idxs_d = nc.dram_tensor("idxs", [128, meta['ICOLS_TOT']], I16, kind="ExternalInput")
    nds_d = nc.dram_tensor("nds", [128, 2 * meta['CH_TOT']], F32, kind="ExternalInput")
    wt_d = nc.dram_tensor("wt", [D, D], F32, kind="ExternalInput")
    bias_d = nc.dram_tensor("bias_row", [1, D], F32, kind="ExternalInput")
    ones_d = nc.dram_tensor("ones_row", [1, D], F32, kind="ExternalInput")
    iota_d = nc.dram_tensor("iota", [128, D], F32, kind="ExternalInput")
    out_d = nc.dram_tensor("out", [SHARD, D], F32, kind="ExternalOutput")

    NG, NP, NF, NEO, TB = cfg.NG, cfg.NP, cfg.NF, cfg.NEO, cfg.TB

    st = ExitStack()
    gS = [st.enter_context(nc.sbuf_tensor(f"g{k}", [128, CHmax, D], F32)) for k in range(NG)]
    iS = [st.enter_context(nc.sbuf_tensor(f"ix{k}", [128, ICOLmax], I16)) for k in range(NG)]
    ndS = [st.enter_context(nc.sbuf_tensor(f"nd{k}", [128, 2 * CHmax], F32)) for k in range(NG)]
    pS = st.enter_context(nc.sbuf_tensor("pring", [128, NP * 128], F32))
    zS = st.enter_context(nc.sbuf_tensor("zring", [128, 2 * 128], F32))
    eoS = [st.enter_context(nc.sbuf_tensor(f"eo{k}", [128, D], F32)) for k in range(NEO)]
    xrS = [st.enter_context(nc.sbuf_tensor(f"xr{k}", [128, D], F32)) for k in range(NEO)]
    wtS = st.enter_context(nc.sbuf_tensor("wts", [D, D], F32))
    biasS = st.enter_context(nc.sbuf_tensor("biass", [1, D], F32))
    onesS = st.enter_context(nc.sbuf_tensor("oness", [1, D], F32))
    iotaS = st.enter_context(nc.sbuf_tensor("iotas", [128, D], F32))

    accum = st.enter_context(nc.psum_tensor("accum", [128, 2 * 512], F32))
    finalP = st.enter_context(nc.psum_tensor("finalp", [128, NF * 512], F32))

    s_idx = [st.enter_context(nc.semaphore(f"s_idx{k}")) for k in range(NG)]
    s_nd = [st.enter_context(nc.semaphore(f"s_nd{k}")) for k in range(NG)]
    s_x = [st.enter_context(nc.semaphore(f"s_x{k}")) for k in range(NEO)]
    s_out = [st.enter_context(nc.semaphore(f"s_out{k}")) for k in range(NEO)]
    s_const = st.enter_context(nc.semaphore("s_const"))
    gsem = [st.enter_context(nc.semaphore(f"gsem{k}")) for k in range(NG)]
    d_chunk = st.enter_context(nc.semaphore("d_chunk"))
    d_z = st.enter_context(nc.semaphore("d_z"))
    d_eo = st.enter_context(nc.semaphore("d_eo"))
    p_chunk = st.enter_context(nc.semaphore("p_chunk"))
    p_final = st.enter_context(nc.semaphore("p_final"))
    d_init = st.enter_context(nc.semaphore("d_init"))

    n_batches = meta['n_batches']
    counts = meta['counts_per_call'][sim_core] if sim_core is not None else None

    # epilogue tile order: tile I done in batch order
    ep_tiles = []
    for b in range(n_batches):
        ep_tiles.extend(range(b * TB, min((b + 1) * TB, TILES)))
    assert ep_tiles == list(range(TILES))

    with nc.Block() as block:

        @block.sync
        def _(sync):
            # consts
            sync.dma_start(wtS[:, :], wt_d[:, :]).then_inc(s_const, 16)
            sync.dma_start(biasS[:, :], bias_d[:, :]).then_inc(s_const, 16)
            sync.dma_start(onesS[:, :], ones_d[:, :]).then_inc(s_const, 16)
            sync.dma_start(iotaS[:, :], iota_d[:, :]).then_inc(s_const, 16)

            def store_tile(t):
                e = t % NEO
                r0 = (t % TILES) * 128
                r1 = min(r0 + 128, SHARD)
                sync.wait_ge(d_eo, t + 1)
                sync.dma_start(out_d[r0:r1, :], eoS[e][:r1 - r0, :]).then_inc(s_out[e], 16)

            def load_xr(t):
                e = t % NEO
                r0 = (t % TILES) * 128
                r1 = min(r0 + 128, SHARD)
                if t >= NEO:
                    sync.wait_ge(d_eo, t - NEO + 1)   # xr slot free
                sync.dma_start(xrS[e][:r1 - r0, :], xshard[r0:r1, :]).then_inc(s_x[e], 16)

            for q, call in enumerate(gsched):
                m = q % NG
                t = call['t']
                if call['w'] == 0 and strip is None:
                    load_xr(t)
                    if t >= 2:
                        store_tile(t - 2)
                dq = call['dram_q']
                ic0, ic1 = int(icol_off[dq]), int(icol_off[dq + 1])
                ch0, ch1 = int(ch_off[dq]), int(ch_off[dq + 1])
                if q >= NG:
                    # idx slot reused after gather of call q-NG completed
                    sync.wait_ge(gsem[m], 16 * (q // NG))
                    # nv/dv slots reused after DVE consumed call q-NG
                    if strip != 'gather':
                        sync.wait_ge(d_chunk, int(cum_chunks[q - NG + 1]))
                sync.dma_start(iS[m][:, :ic1 - ic0], idxs_d[:, ic0:ic1]).then_inc(s_idx[m], 16)
                sync.dma_start(ndS[m][:, :2 * (ch1 - ch0)], nds_d[:, 2 * ch0:2 * ch1]).then_inc(s_nd[m], 16)
            for t in (range(max(0, GT - 2), GT) if strip is None else []):
                store_tile(t)
            for e in (range(NEO) if strip is None else []):
                uses = len([t for t in range(GT) if t % NEO == e])
                if uses:
                    sync.wait_ge(s_out[e], 16 * uses)

        @block.gpsimd
        def _(gpsimd):
            gpsimd.load_library(mlp)
            gpsimd.wait_ge(d_init, NG)
            for q, call in enumerate(gsched):
                m = q % NG
                cap, CH, w = call['cap'], call['CH'], call['w']
                gpsimd.wait_ge(s_idx[m], 16 * (q // NG + 1))
                if q >= NG:
                    if strip == 'gather':
                        gpsimd.wait_ge(gsem[m], 16 * (q // NG))
                    elif strip == 'nope':
                        gpsimd.wait_ge(d_chunk, int(cum_chunks[q - NG + 1]))
                    else:
                        gpsimd.wait_ge(p_chunk, int(cum_chunks[q - NG + 1]))
                w0 = w * WIN
                w1 = min(w0 + WIN, N)
                nreg = int(meta['vq'][call['dram_q']])
                gpsimd.dma_gather(
                    gS[m][:, :CH, :], xfull[w0:w1, :], iS[m][:, :cap // 16],
                    cap, nreg, D, single_packet=False, queue_num=m % 4,
                ).then_inc(gsem[m], 16)

        @block.vector
        def _(vector):
            for k in range(NG):
                vector.memset(gS[k][:, :, :], 0.0).then_inc(d_init, 1)
            vector.wait_ge(s_const, 64)

            def epilogue(t):
                f = t % NF
                e = t % NEO
                vector.wait_ge(p_final, t + 1)
                vector.wait_ge(s_x[e], 16 * (t // NEO + 1))
                if t >= NEO:
                    vector.wait_ge(s_out[e], 16 * (t // NEO))  # eo slot free
                vector.scalar_tensor_tensor(
                    eoS[e][:, :], finalP[:, f * 512:f * 512 + 128], 0.0,
                    xrS[e][:, :], ALU.max, ALU.add,
                ).then_inc(d_eo, 1)

            if strip == 'gather':
                return
            for q, call in enumerate(gsched):
                m = q % NG
                CH, t = call['CH'], call['t']
                vector.wait_ge(s_nd[m], 16 * (q // NG + 1))
                for j in range(CH):
                    g = int(cum_chunks[q]) + j
                    p = g % NP
                    if g >= NP and strip != 'nope':
                        vector.wait_ge(p_chunk, g - NP + 1)
                    vector.tensor_scalar(
                        pS[:, p * 128:(p + 1) * 128], iotaS[:, :],
                        ndS[m][:, CH + j:CH + j + 1], ndS[m][:, j:j + 1],
                        ALU.is_equal, ALU.mult,
                    ).then_inc(d_chunk, 1)
                if call['last'] and strip is None:
                    # copy this tile's accumulated Z out of PSUM
                    z = t % 2
                    vector.wait_ge(p_chunk, tile_last_chunk[t] + 1)
                    if t >= 2:
                        vector.wait_ge(p_final, t - 1)  # zS slot free
                    vector.tensor_copy(
                        zS[:, z * 128:(z + 1) * 128],
                        accum[:, z * 512:z * 512 + 128],
                    ).then_inc(d_z, 1)
                    if t >= 1:
                        epilogue(t - 1)
            if strip is None:
                epilogue(GT - 1)

        @block.tensor
        def _(tensor):
            if strip in ('gather', 'nope'):
                return
            tensor.wait_ge(s_const, 64)

            def finals(t):
                z = t % 2
                f = t % NF
                tensor.wait_ge(d_z, t + 1)
                if t >= NF:
                    tensor.wait_ge(d_eo, t - NF + 1)
                tensor.matmul(
                    finalP[:, f * 512:f * 512 + 128],
                    zS[:, z * 128:(z + 1) * 128], wtS[:, :],
                    start=True, stop=False, skip_group_check=True,
                )
                tensor.matmul(
                    finalP[:, f * 512:f * 512 + 128],
                    onesS[:1, :], biasS[:1, :],
                    start=False, stop=True, skip_group_check=True,
                ).then_inc(p_final, 1)

            for q, call in enumerate(gsched):
                m = q % NG
                CH, t = call['CH'], call['t']
                tensor.wait_ge(gsem[m], 16 * (q // NG + 1))
                for j in range(CH):
                    g = int(cum_chunks[q]) + j
                    p = g % NP
                    tensor.wait_ge(d_chunk, g + 1)
                    is_first = call['first'] and j == 0
                    is_last = call['last'] and j == CH - 1
                    tensor.matmul(
                        accum[:, (t % 2) * 512:(t % 2) * 512 + 128],
                        gS[m][:, j, :], pS[:, p * 128:(p + 1) * 128],
                        start=is_first, stop=is_last, skip_group_check=True,
                    ).then_inc(p_chunk, 1)
                # finals of previous tile overlap this tile's chunks
                if call['first'] and t >= 1 and strip is None:
                    finals(t - 1)
            if strip is None:
                finals(GT - 1)

    st.close()
    nc.compile()
    return nc


def reference_np(x, weight, bias, edge_weight, edge_index):
    N = x.shape[0]
    src = np.asarray(edge_index[0], dtype=np.int64)
    dst = np.asarray(edge_index[1], dtype=np.int64)
    ew = np.asarray(edge_weight, dtype=np.float64)
    deg = np.bincount(dst, weights=ew, minlength=N) + 1.0
    dinv = 1.0 / np.sqrt(deg)
    h = x.astype(np.float64) @ np.asarray(weight, dtype=np.float64).T
    nrm = dinv[src] * ew * dinv[dst]
    msg = h[src] * nrm[:, None]
    out = np.zeros_like(h)
    np.add.at(out, dst, msg)
    out += (dinv * dinv)[:, None] * h
    out = out + np.asarray(bias, dtype=np.float64)
    out = np.maximum(out, 0.0) + x.astype(np.float64)
    return out


_CFG = Cfg(100000, 3200000, WIN=32768, NG=8, NP=32, NF=4, NEO=4)


def kernel(x, weight, bias, edge_weight, edge_index):
    """GCN block on 8 Trainium2 NeuronCores. Full inputs in, full output out."""
    from concourse.bass_utils import run_bass_kernel_spmd

    x = np.ascontiguousarray(np.asarray(x, dtype=np.float32))
    weight = np.asarray(weight, dtype=np.float32)
    bias = np.asarray(bias, dtype=np.float32)
    edge_weight = np.asarray(edge_weight, dtype=np.float32)
    edge_index = np.asarray(edge_index)

    meta, in_maps = prep(_CFG, x, weight, bias, edge_weight, edge_index)
    nc = build(_CFG, meta)
    res = run_bass_kernel_spmd(nc, in_maps, list(range(_CFG.NC)))
    out = np.concatenate([res.results[c]["out"] for c in range(_CFG.NC)], axis=0)
    return out.astype(np.float32)



# revision 3
# speedup vs baseline: 1.4549x; 1.4549x over previous
"""GCN block kernel for TRN2, 8-core SPMD — bf16 gather pipeline.

Algorithm (per core, destination-sharded):
  out[dst] = relu( (sum_e norm_e * x[src_e]) @ W.T + bias ) + x[dst]
Edge aggregation runs on bf16 x rows (dma_gather), accumulated per 128-row
destination tile via one-hot bf16 matmuls on the PE; one 128x128 bf16 weight
matmul per destination tile finishes the job (bias folded as rank-1 matmul).

Vs v1: bf16 everywhere on the edge path (half the gather bytes, 4x-mode DVE
one-hot builds, FWL bf16 matmuls), gather calls batched over TB=8 dst tiles
(52 calls/core instead of 392 — amortizes the ~1us SWDGE fixed cost),
call-granular semaphores, PSUM->SBUF copies moved to the ACT engine.

Edge partitioning: by (dst tile, source window of 25000 rows); per-(tile,win)
capacity = max-over-cores count rounded to 128 (uniform SPMD schedule);
padding slots gather row 0 of the window with norm 0.
"""
import sys
sys.path.insert(0, '/opt/trn_rl_repo')
import numpy as np
from contextlib import ExitStack

import concourse.bacc as bacc
import concourse.mybir as mybir
from concourse.library_config import mlp

F32 = mybir.dt.float32
BF16 = mybir.dt.bfloat16
I16 = mybir.dt.int16
ALU = mybir.AluOpType
ACTF = mybir.ActivationFunctionType

D = 128


def roundup(x, m):
    return (x + m - 1) // m * m


class Cfg:
    def __init__(self, N, E, NC=8, WIN=32768, TB=4, NG=8, NPC=4, NZ=4,
                 NXR=16, NEOS=16):
        self.N, self.E, self.NC = N, E, NC
        assert N % NC == 0
        self.SHARD = N // NC
        self.TILES = (self.SHARD + 127) // 128
        self.WIN = WIN
        self.NW = (N + WIN - 1) // WIN
        self.TB = TB        # tiles per batch; 2*TB PSUM banks (1 per tile)
        assert 2 * TB <= 8
        self.NB = (self.TILES + TB - 1) // TB   # batches
        self.NG = NG        # gather / idx ring depth (call slots)
        self.NPC = NPC      # P ring (call slots)
        self.NZ = NZ        # zS ring (tiles)
        self.NXR = NXR      # x-residual ring (tiles)
        self.NEOS = NEOS    # epilogue-out ring (tiles)


def prep(cfg, x, weight, bias, edge_weight, edge_index):
    """Host preprocessing -> (meta, per-core in_maps)."""
    N, NC, SHARD, TILES, WIN, NW, TB, NB = (
        cfg.N, cfg.NC, cfg.SHARD, cfg.TILES, cfg.WIN, cfg.NW, cfg.TB, cfg.NB)
    src = np.asarray(edge_index[0], dtype=np.int64)
    dst = np.asarray(edge_index[1], dtype=np.int64)
    ew = np.asarray(edge_weight, dtype=np.float64)
    deg = np.bincount(dst, weights=ew, minlength=N) + 1.0
    dinv = (1.0 / np.sqrt(deg)).astype(np.float32)
    norm = (dinv[src] * ew.astype(np.float32) * dinv[dst]).astype(np.float32)

    loop = np.arange(N, dtype=np.int64)
    a_src = np.concatenate([src, loop])
    a_dst = np.concatenate([dst, loop])
    a_nrm = np.concatenate([norm, (dinv * dinv).astype(np.float32)])

    core = a_dst // SHARD
    dstloc = a_dst - core * SHARD
    tile = dstloc // 128
    col = (dstloc % 128).astype(np.float32)
    win = a_src // WIN
    srcloc = (a_src - win * WIN).astype(np.int32)

    key = (core * TILES + tile) * NW + win
    order = np.argsort(key, kind='stable')
    counts = np.bincount(key, minlength=NC * TILES * NW).reshape(NC, TILES, NW)
    starts = np.zeros(NC * TILES * NW + 1, dtype=np.int64)
    np.cumsum(counts.reshape(-1), out=starts[1:])

    s_srcloc = srcloc[order]
    s_nrm = a_nrm[order]
    s_col = col[order]

    # per-(tile, win) capacity: max over cores, rounded to a 128-chunk
    tcap = np.maximum(128, ((counts.max(axis=0) + 127) // 128) * 128)  # [TILES, NW]

    # schedule: batches of TB tiles; per batch, NW window calls
    batches = [list(range(b * TB, min((b + 1) * TB, TILES))) for b in range(NB)]
    sched = []
    for b, btiles in enumerate(batches):
        for w in range(NW):
            members = []
            off = 0
            for t in btiles:
                cap_t = int(tcap[t, w])
                members.append(dict(tile=t, cap=cap_t, coff=off // 128))
                off += cap_t
            sched.append(dict(b=b, w=w, tiles=btiles, members=members,
                              cap=off, CH=off // 128,
                              first=(w == 0), last=(w == NW - 1)))

    icols = [c['cap'] // 16 for c in sched]
    chs = [c['CH'] for c in sched]
    icol_off = np.concatenate([[0], np.cumsum(icols)])
    ch_off = np.concatenate([[0], np.cumsum(chs)])
    ICOLS_TOT, CH_TOT = int(icol_off[-1]), int(ch_off[-1])

    idx_streams, nd_streams = [], []
    for c in range(NC):
        idx_s = np.zeros((16, ICOLS_TOT), dtype=np.int16)
        nd_s = np.zeros((128, 2 * CH_TOT), dtype=np.float32)
        for q, call in enumerate(sched):
            w = call['w']
            ipad = np.zeros(call['cap'], dtype=np.int16)
            npad = np.zeros(call['cap'], dtype=np.float32)
            cpad = np.zeros(call['cap'], dtype=np.float32)
            off = 0
            for mem in call['members']:
                t = mem['tile']
                g = (c * TILES + t) * NW + w
                lo, hi = int(starts[g]), int(starts[g + 1])
                cnt = hi - lo
                assert cnt <= mem['cap']
                # ascending source order within the group: better HBM row
                # locality for the gather
                sub = np.argsort(s_srcloc[lo:hi], kind='stable')
                ipad[off:off + cnt] = s_srcloc[lo:hi][sub].astype(np.int16)
                npad[off:off + cnt] = s_nrm[lo:hi][sub]
                cpad[off:off + cnt] = s_col[lo:hi][sub]
                off += mem['cap']
            idx_s[:, icol_off[q]:icol_off[q + 1]] = ipad.reshape(-1, 16).T
            CH = call['CH']
            c0 = 2 * ch_off[q]
            nd_s[:, c0:c0 + CH] = npad.reshape(-1, 128).T
            nd_s[:, c0 + CH:c0 + 2 * CH] = cpad.reshape(-1, 128).T
        idx_streams.append(np.tile(idx_s, (8, 1)))
        nd_streams.append(nd_s)

    meta = dict(sched=sched, icol_off=icol_off, ch_off=ch_off,
                ICOLS_TOT=ICOLS_TOT, CH_TOT=CH_TOT,
                CHmax=max(chs), ICOLmax=max(icols))

    import ml_dtypes
    bf = ml_dtypes.bfloat16
    wt = np.ascontiguousarray(np.asarray(weight, dtype=np.float32).T.astype(bf))
    bias_row = np.asarray(bias, dtype=np.float32).reshape(1, D).astype(bf)
    ones_row = np.ones((1, D), dtype=bf)
    iota = np.tile(np.arange(D, dtype=np.float32), (128, 1)).astype(bf)

    xf = np.asarray(x, dtype=np.float32)
    xbf = xf.astype(bf)
    in_maps = []
    for c in range(NC):
        in_maps.append({
            "xbf": xbf,
            "xshard": np.ascontiguousarray(xf[c * SHARD:(c + 1) * SHARD]),
            "idxs": idx_streams[c],
            "nds": nd_streams[c],
            "wt": wt, "bias_row": bias_row, "ones_row": ones_row, "iota": iota,
        })
    return meta, in_maps


def build(cfg, meta, reps=1, strip=None, single_packet=False):
    """strip: None (full) | 'gather' (SP+Pool) | 'dve' (+P builds) |
    'pe' (+chunk matmuls, no finals/epilogue/stores)."""
    N, SHARD, TILES, WIN, NW, TB, NB = (
        cfg.N, cfg.SHARD, cfg.TILES, cfg.WIN, cfg.NW, cfg.TB, cfg.NB)
    sched, icol_off, ch_off = meta['sched'], meta['icol_off'], meta['ch_off']
    NCALLS = len(sched)
    CHmax, ICOLmax = meta['CHmax'], meta['ICOLmax']
    NG, NPC, NZ, NXR, NEOS = cfg.NG, cfg.NPC, cfg.NZ, cfg.NXR, cfg.NEOS

    # global schedule across reps
    gsched = []
    for r in range(reps):
        for q, call in enumerate(sched):
            gsched.append(dict(call, dram_q=q, gb=r * NB + call['b'],
                               gtiles=[r * TILES + t for t in call['tiles']]))
    GQ = len(gsched)
    GT = reps * TILES
    GB = reps * NB

    # per-batch tile lists (global), and per-tile (batch, slot) -> PSUM bank.
    # Each open accumulator tile owns one full PSUM bank: bank (gb%2)*TB + k.
    # Z accumulates in cols [0:128) of the bank; the finals matmul writes its
    # result into cols [128:256) of the same bank (sem-ordered after the ACT
    # copy, so no cross-engine same-bank access is ever concurrent).
    gbatch_tiles = {}
    tile_bank = {}
    for call in gsched:
        gbatch_tiles[call['gb']] = call['gtiles']
        for k, gt in enumerate(call['gtiles']):
            tile_bank[gt] = (call['gb'] % 2) * TB + k

    nc = bacc.Bacc("TRN2", num_swdge_queues=4)

    xbf_d = nc.dram_tensor("xbf", [N, D], BF16, kind="ExternalInput")
    xshard_d = nc.dram_tensor("xshard", [SHARD, D], F32, kind="ExternalInput")
    idxs_d = nc.dram_tensor("idxs", [128, meta['ICOLS_TOT']], I16,
                            kind="ExternalInput")
    nds_d = nc.dram_tensor("nds", [128, 2 * meta['CH_TOT']], F32,
                           kind="ExternalInput")
    wt_d = nc.dram_tensor("wt", [D, D], BF16, kind="ExternalInput")
    bias_d = nc.dram_tensor("bias_row", [1, D], BF16, kind="ExternalInput")
    ones_d = nc.dram_tensor("ones_row", [1, D], BF16, kind="ExternalInput")
    iota_d = nc.dram_tensor("iota", [128, D], BF16, kind="ExternalInput")
    out_d = nc.dram_tensor("out", [SHARD, D], F32, kind="ExternalOutput")

    st = ExitStack()
    gS = [st.enter_context(nc.sbuf_tensor(f"g{k}", [128, CHmax, D], BF16))
          for k in range(NG)]
    iS = [st.enter_context(nc.sbuf_tensor(f"ix{k}", [128, ICOLmax], I16))
          for k in range(NG)]
    ndS = [st.enter_context(nc.sbuf_tensor(f"nd{k}", [128, 2 * CHmax], F32))
           for k in range(NG)]
    pS = st.enter_context(nc.sbuf_tensor("pring", [128, NPC * CHmax * 128], BF16))
    zS = st.enter_context(nc.sbuf_tensor("zring", [128, NZ * 128], BF16))
    eoS = [st.enter_context(nc.sbuf_tensor(f"eo{k}", [128, D], F32))
           for k in range(NEOS)]
    xrS = [st.enter_context(nc.sbuf_tensor(f"xr{k}", [128, D], F32))
           for k in range(NXR)]
    wtS = st.enter_context(nc.sbuf_tensor("wts", [D, D], BF16))
    biasS = st.enter_context(nc.sbuf_tensor("biass", [1, D], BF16))
    onesS = st.enter_context(nc.sbuf_tensor("oness", [1, D], BF16))
    iotaS = st.enter_context(nc.sbuf_tensor("iotas", [128, D], BF16))

    accum = st.enter_context(nc.psum_tensor("accum", [128, 2 * TB * 512], F32))

    s_idx = [st.enter_context(nc.semaphore(f"s_idx{k}")) for k in range(NG)]
    s_nd = [st.enter_context(nc.semaphore(f"s_nd{k}")) for k in range(NG)]
    s_x = [st.enter_context(nc.semaphore(f"s_x{k}")) for k in range(NXR)]
    s_out = [st.enter_context(nc.semaphore(f"s_out{k}")) for k in range(NEOS)]
    s_const = st.enter_context(nc.semaphore("s_const"))
    gsem = [st.enter_context(nc.semaphore(f"gsem{k}")) for k in range(NG)]
    d_call = st.enter_context(nc.semaphore("d_call"))    # DVE per call
    p_call = st.enter_context(nc.semaphore("p_call"))    # PE per call
    d_z = st.enter_context(nc.semaphore("d_z"))          # ACT per tile
    p_final = st.enter_context(nc.semaphore("p_final"))  # PE per tile
    d_eo = st.enter_context(nc.semaphore("d_eo"))        # DVE per tile

    def tile_rows(gt):
        r0 = (gt % TILES) * 128
        r1 = min(r0 + 128, SHARD)
        return r0, r1

    with nc.Block() as block:

        @block.sync
        def _(sync):
            sync.dma_start(wtS[:, :], wt_d[:, :]).then_inc(s_const, 16)
            sync.dma_start(biasS[:, :], bias_d[:, :]).then_inc(s_const, 16)
            sync.dma_start(onesS[:, :], ones_d[:, :]).then_inc(s_const, 16)
            sync.dma_start(iotaS[:, :], iota_d[:, :]).then_inc(s_const, 16)

            def load_xr(gt):
                e = gt % NXR
                r0, r1 = tile_rows(gt)
                if gt >= NXR:
                    sync.wait_ge(d_eo, gt - NXR + 1)
                sync.dma_start(xrS[e][:r1 - r0, :],
                               xshard_d[r0:r1, :]).then_inc(s_x[e], 16)

            def store_tile(gt):
                e = gt % NEOS
                r0, r1 = tile_rows(gt)
                sync.wait_ge(d_eo, gt + 1)
                sync.dma_start(out_d[r0:r1, :],
                               eoS[e][:r1 - r0, :]).then_inc(s_out[e], 16)

            for q, call in enumerate(gsched):
                m = q % NG
                if call['first'] and strip is None:
                    for gt in call['gtiles']:
                        load_xr(gt)
                    # stores for batch gb-2
                    if call['gb'] >= 2:
                        for gt in gbatch_tiles[call['gb'] - 2]:
                            store_tile(gt)
                dq = call['dram_q']
                ic0, ic1 = int(icol_off[dq]), int(icol_off[dq + 1])
                ch0, ch1 = int(ch_off[dq]), int(ch_off[dq + 1])
                if q >= NG:
                    # idx slot free once gather of call q-NG ran
                    sync.wait_ge(gsem[m], 16 * (q // NG))
                    # nd slot free once DVE consumed call q-NG
                    if strip != 'gather':
                        sync.wait_ge(d_call, q - NG + 1)
                sync.dma_start(iS[m][:, :ic1 - ic0],
                               idxs_d[:, ic0:ic1]).then_inc(s_idx[m], 16)
                sync.dma_start(ndS[m][:, :2 * (ch1 - ch0)],
                               nds_d[:, 2 * ch0:2 * ch1]).then_inc(s_nd[m], 16)
            if strip is None:
                for gb in range(max(0, GB - 2), GB):
                    for gt in gbatch_tiles[gb]:
                        store_tile(gt)
                for e in range(NEOS):
                    uses = len([t for t in range(GT) if t % NEOS == e])
                    if uses:
                        sync.wait_ge(s_out[e], 16 * uses)
            elif strip == 'gather':
                for m in range(NG):
                    uses = len([q for q in range(GQ) if q % NG == m])
                    sync.wait_ge(gsem[m], 16 * uses)
            elif strip == 'dve':
                sync.wait_ge(d_call, GQ)
            elif strip == 'pe':
                sync.wait_ge(p_call, GQ)

        @block.gpsimd
        def _(gpsimd):
            gpsimd.load_library(mlp)
            for q, call in enumerate(gsched):
                m = q % NG
                cap, w = call['cap'], call['w']
                gpsimd.wait_ge(s_idx[m], 16 * (q // NG + 1))
                if q >= NG:
                    if strip == 'gather':
                        gpsimd.wait_ge(gsem[m], 16 * (q // NG))
                    elif strip == 'dve':
                        gpsimd.wait_ge(d_call, q - NG + 1)
                    else:
                        gpsimd.wait_ge(p_call, q - NG + 1)  # gS slot free
                w0 = w * WIN
                w1 = min(w0 + WIN, N)
                gpsimd.dma_gather(
                    gS[m][:, :call['CH'], :], xbf_d[w0:w1, :],
                    iS[m][:, :cap // 16], cap, cap, D,
                    single_packet=single_packet, queue_num=q % 4,
                ).then_inc(gsem[m], 16)

        @block.vector
        def _(vector):
            if strip == 'gather':
                return
            vector.wait_ge(s_const, 64)

            def epilogue(gt):
                bk = tile_bank[gt]
                e = gt % NEOS
                xe = gt % NXR
                vector.wait_ge(p_final, gt + 1)
                vector.wait_ge(s_x[xe], 16 * (gt // NXR + 1))
                if gt >= NEOS:
                    vector.wait_ge(s_out[e], 16 * (gt // NEOS))
                vector.scalar_tensor_tensor(
                    eoS[e][:, :], accum[:, bk * 512 + 128:bk * 512 + 256], 0.0,
                    xrS[xe][:, :], ALU.max, ALU.add,
                ).then_inc(d_eo, 1)

            for q, call in enumerate(gsched):
                m = q % NG
                CH = call['CH']
                s = q % NPC
                vector.wait_ge(s_nd[m], 16 * (q // NG + 1))
                if q >= NPC and strip not in ('dve',):
                    vector.wait_ge(p_call, q - NPC + 1)  # pS slot free
                for j in range(CH):
                    ins = vector.tensor_scalar(
                        pS[:, (s * CHmax + j) * 128:(s * CHmax + j + 1) * 128],
                        iotaS[:, :],
                        ndS[m][:, CH + j:CH + j + 1],
                        ndS[m][:, j:j + 1],
                        ALU.is_equal, ALU.mult,
                    )
                    if j == CH - 1:
                        ins.then_inc(d_call, 1)
                # epilogues for batch gb-1 spread across this batch's calls
                if strip is None and call['gb'] >= 1 and not call['first']:
                    prev = gbatch_tiles[call['gb'] - 1]
                    w = call['w']
                    lo = (len(prev) * (w - 1)) // (NW - 1)
                    hi = (len(prev) * w) // (NW - 1)
                    for gt in prev[lo:hi]:
                        epilogue(gt)
            if strip is None:
                for gt in gbatch_tiles[GB - 1]:
                    epilogue(gt)

        @block.scalar
        def _(scalar):
            if strip is not None:
                return
            # PSUM accum -> SBUF zS (bf16 cast) per tile, after PE finishes
            # the batch's last window call
            for gb in range(GB):
                last_q = gb * NW + (NW - 1)
                for k, gt in enumerate(gbatch_tiles[gb]):
                    if k == 0:
                        scalar.wait_ge(p_call, last_q + 1)
                    if gt >= NZ:
                        scalar.wait_ge(p_final, gt - NZ + 1)  # zS slot free
                    bk = tile_bank[gt]
                    scalar.activation(
                        zS[:, (gt % NZ) * 128:(gt % NZ + 1) * 128],
                        accum[:, bk * 512:bk * 512 + 128],
                        ACTF.Copy,
                    ).then_inc(d_z, 1)

        @block.tensor
        def _(tensor):
            if strip in ('gather', 'dve'):
                return
            tensor.wait_ge(s_const, 64)

            def finals(gt):
                z = gt % NZ
                bk = tile_bank[gt]
                fr = accum[:, bk * 512 + 128:bk * 512 + 256]
                tensor.wait_ge(d_z, gt + 1)
                tensor.matmul(
                    fr, zS[:, z * 128:(z + 1) * 128], wtS[:, :],
                    start=True, stop=False, skip_group_check=True,
                )
                tensor.matmul(
                    fr, onesS[:1, :], biasS[:1, :],
                    start=False, stop=True, skip_group_check=True,
                ).then_inc(p_final, 1)

            for q, call in enumerate(gsched):
                m = q % NG
                s = q % NPC
                gb = call['gb']
                tensor.wait_ge(gsem[m], 16 * (q // NG + 1))
                tensor.wait_ge(d_call, q + 1)
                if call['first'] and gb >= 2 and strip is None:
                    # this batch's banks were last read by the epilogues of
                    # batch gb-2; d_eo counts tiles in order, so wait for
                    # everything before batch gb-1's first tile
                    tensor.wait_ge(d_eo, gbatch_tiles[gb - 1][0])
                for mem_i, mem in enumerate(call['members']):
                    gt = call['gtiles'][mem_i]
                    bk = tile_bank[gt]
                    nch = mem['cap'] // 128
                    for jj in range(nch):
                        j = mem['coff'] + jj
                        ins = tensor.matmul(
                            accum[:, bk * 512:bk * 512 + 128],
                            gS[m][:, j, :],
                            pS[:, (s * CHmax + j) * 128:(s * CHmax + j + 1) * 128],
                            start=(call['first'] and jj == 0),
                            stop=(call['last'] and jj == nch - 1),
                            skip_group_check=True,
                        )
                        if mem_i == len(call['members']) - 1 and jj == nch - 1:
                            ins.then_inc(p_call, 1)
                # finals for batch gb-1 spread across this batch's calls
                if strip is None and gb >= 1 and not call['last']:
                    prev = gbatch_tiles[gb - 1]
                    w = call['w']
                    lo = (len(prev) * w) // (NW - 1)
                    hi = (len(prev) * (w + 1)) // (NW - 1)
                    for gt in prev[lo:hi]:
                        finals(gt)
            if strip is None:
                for gt in gbatch_tiles[GB - 1]:
                    finals(gt)

    st.close()
    nc.compile()
    return nc


def reference_np(x, weight, bias, edge_weight, edge_index):
    N = x.shape[0]
    src = np.asarray(edge_index[0], dtype=np.int64)
    dst = np.asarray(edge_index[1], dtype=np.int64)
    ew = np.asarray(edge_weight, dtype=np.float64)
    deg = np.bincount(dst, weights=ew, minlength=N) + 1.0
    dinv = 1.0 / np.sqrt(deg)
    h = x.astype(np.float64) @ np.asarray(weight, dtype=np.float64).T
    nrm = dinv[src] * ew * dinv[dst]
    msg = h[src] * nrm[:, None]
    out = np.zeros_like(h)
    np.add.at(out, dst, msg)
    out += (dinv * dinv)[:, None] * h
    out = out + np.asarray(bias, dtype=np.float64)
    out = np.maximum(out, 0.0) + x.astype(np.float64)
    return out


_CFG = Cfg(100000, 3200000)


def kernel(x, weight, bias, edge_weight, edge_index):
    """GCN block on 8 Trainium2 NeuronCores. Full inputs in, full output out."""
    from concourse.bass_utils import run_bass_kernel_spmd

    x = np.ascontiguousarray(np.asarray(x, dtype=np.float32))
    weight = np.asarray(weight, dtype=np.float32)
    bias = np.asarray(bias, dtype=np.float32)
    edge_weight = np.asarray(edge_weight, dtype=np.float32)
    edge_index = np.asarray(edge_index)

    meta, in_maps = prep(_CFG, x, weight, bias, edge_weight, edge_index)
    nc = build(_CFG, meta)
    res = run_bass_kernel_spmd(nc, in_maps, list(range(_CFG.NC)))
    out = np.concatenate([res.results[c]["out"] for c in range(_CFG.NC)], axis=0)
    return out.astype(np.float32)
